# revision 1
# baseline (speedup 1.0000x reference)
"""HDTimeCrystalBlock kernel for 8 Trainium2 NeuronCores.

Math: out = ((x @ W_in) * mod[None]) @ W_out, where
  mod[l,h] = sum_m coupled[m] * cos(omega*(m+1)*t[l] + E[m,h])
Using cos(a+b) = cos(a)cos(b) - sin(a)sin(b):
  mod = C @ A + S @ B,  A[m,h] = coupled[m]*cos(E[m,h]),
                        B[m,h] = -coupled[m]*sin(E[m,h]),
  C[l,m] = cos(omega*(m+1)*t[l]), S[l,m] = sin(...)
so the [L,M,HD] cos tensor never materializes: mod is a K=2M matmul.

Sharding: data-parallel over the 8192 tokens (B*L), 1024 per core; weights
replicated. Activations stay transposed ([feature, token]) on device so both
matmuls consume natural-layout weights as the stationary operand; the host
does the x/y transposes. Matmuls run in float32r (full PE rate, ~2e-4 rel
err). E's cos/sin run on device with sign-based range reduction to [-pi,pi]
(ACT Sin LUT is only accurate there). Weight DMAs are chunked and ordered so
the PE starts within a few microseconds and never starves (HAM stays warm).
"""
import math

import numpy as np

B, L, D, HD, M = 4, 2048, 512, 4096, 16
NCORES = 8
T = (B * L) // NCORES          # tokens per core
QCH = 512                      # l-chunk (PSUM bank width in fp32)
NQ = T // QCH
NJ = HD // 128                 # h-tiles
NK = D // 128                  # d-tiles
NC_ = HD // 1024               # w_in column chunks
JPC = 1024 // 128              # j-tiles per w_in chunk
PI = math.pi

_cache = {}


USE_BF16 = True


def _build():
    from concourse import bacc, bass, mybir, tile

    F32 = mybir.dt.float32
    F32R = mybir.dt.float32r
    MMD = mybir.dt.bfloat16 if USE_BF16 else F32R
    AF = mybir.ActivationFunctionType
    PSUM = bass.MemorySpace.PSUM

    nc = bacc.Bacc("TRN2", target_bir_lowering=False, debug=False)

    xT_d = nc.dram_tensor("xT", [D, T], MMD, kind="ExternalInput")
    w_in_d = nc.dram_tensor("w_in", [D, HD], MMD, kind="ExternalInput")
    w_out_d = nc.dram_tensor("w_out", [HD, D], MMD, kind="ExternalInput")
    cs_d = nc.dram_tensor("cs", [128, T], MMD, kind="ExternalInput")
    fe_d = nc.dram_tensor("fe", [128, M * HD // 128], F32, kind="ExternalInput")
    cbn_d = nc.dram_tensor("cbn", [128, 1], F32, kind="ExternalInput")
    yT_d = nc.dram_tensor("yT", [D, T], F32, kind="ExternalOutput")

    FEW = M * HD // 128        # 512

    with tile.TileContext(nc) as tc:
        with (
            tc.tile_pool(name="win", bufs=1) as winp,
            tc.tile_pool(name="wout", bufs=1) as woutp,
            tc.tile_pool(name="xts", bufs=1) as xtp,
            tc.tile_pool(name="small", bufs=1) as smallp,
            tc.tile_pool(name="prep", bufs=1) as prepp,
            tc.tile_pool(name="hm", bufs=3) as hmp,
            tc.tile_pool(name="mods", bufs=3) as modsp,
            tc.tile_pool(name="yo", bufs=2) as yop,
            tc.tile_pool(name="pa", bufs=2, space=PSUM) as pap,
            tc.tile_pool(name="pb", bufs=2, space=PSUM) as pbp,
            tc.tile_pool(name="py", bufs=4, space=PSUM) as pyp,
        ):
            # ---- tiny inputs first (they gate the mod path) ----
            fe = prepp.tile([128, FEW], F32, tag="fe")
            cbn = smallp.tile([128, 1], F32, tag="cbn")
            cs = smallp.tile([128, T], MMD, tag="cs")
            nc.sync.dma_start(fe[:], fe_d[:])
            nc.sync.dma_start(cbn[:], cbn_d[:])
            nc.sync.dma_start(cs[:], cs_d[:])
            wm = smallp.tile([128, 128], MMD, tag="wm")
            nc.sync.dma_start(wm[:], w_in_d[0:128, 0:128])

            # ---- E -> A=(cb*cosE), B=(-cb*sinE), in [128,512] layout ----
            # bias-free formulation (only 0.0-bias activations + one Copy):
            #   Er   = E - 2*pi*sign(E)*step(|E|-pi)       in [-pi, pi]
            #   sinE = Sin(Er)
            #   -cosE = Sin(|Er| - pi/2)
            #   A = (-cb) * (-cosE),  B = (-cb) * sinE
            sgn = prepp.tile([128, FEW], F32, tag="sgn")
            wk = prepp.tile([128, FEW], F32, tag="wk")
            er = prepp.tile([128, FEW], F32, tag="er")
            nc.scalar.activation(sgn[:], fe[:], AF.Sign)            # sign(E)
            nc.scalar.activation(wk[:], fe[:], AF.Abs)              # |E|
            nc.vector.tensor_scalar_sub(wk[:], wk[:], PI)           # |E|-pi
            nc.scalar.activation(wk[:], wk[:], AF.Sign)             # sign(|E|-pi)
            nc.scalar.activation(wk[:], wk[:], AF.Copy, bias=PI, scale=PI)
            nc.vector.tensor_mul(wk[:], wk[:], sgn[:])              # {0,±2pi}
            nc.vector.tensor_sub(er[:], fe[:], wk[:])               # Er
            nc.scalar.activation(sgn[:], er[:], AF.Sin)             # sin(E)
            nc.scalar.activation(wk[:], er[:], AF.Abs)              # |Er|
            nc.vector.tensor_scalar_sub(wk[:], wk[:], PI / 2)       # |Er|-pi/2
            nc.scalar.activation(er[:], wk[:], AF.Sin)              # -cos(E)
            a128 = prepp.tile([128, FEW], MMD, tag="a128")
            b128 = prepp.tile([128, FEW], MMD, tag="b128")
            nc.vector.tensor_scalar_mul(a128[:], er[:], cbn[:, 0:1])   # A
            nc.vector.tensor_scalar_mul(b128[:], sgn[:], cbn[:, 0:1])  # B

            # ---- reshape A,B [128,512] -> ab rows 0:2M; rows 2M:128 are
            # zero so pb runs as a uniform K=128 matmul ----
            ab = smallp.tile([128, HD], MMD, tag="ab")
            for zp in range(2 * M, 128, 32):
                nc.gpsimd.memset(ab[zp : zp + 32, :], 0.0)
            nc.gpsimd.dma_start(ab[0:M, :], a128[:, :])
            nc.gpsimd.dma_start(ab[M : 2 * M, :], b128[:, :])

            # ---- bulk loads: few large coalesced DMAs, ordered so the
            # PE starts early and never starves ----
            # win_c[c]: [128, NK, 1024] -- all 4 d-tiles of h-chunk c
            w_in_r = w_in_d.ap().rearrange("(k p) (c h) -> c p k h", p=128, c=NC_)
            xT_r = xT_d.ap().rearrange("(k p) (q t) -> q p k t", p=128, q=NQ)
            w_out_r = w_out_d.ap().rearrange("(g jj p) i -> g p jj i", p=128, jj=JPC)

            win_c = [None] * NC_
            xts_q = [None] * NQ
            wout_g = [None] * NC_

            def load_win(c):
                t_ = winp.tile([128, NK, 1024], MMD, name=f"win{c}", tag=f"win{c}")
                nc.sync.dma_start(t_[:], w_in_r[c])
                win_c[c] = t_

            def load_xts(q):
                tx = xtp.tile([128, NK, QCH], MMD, name=f"xts{q}", tag=f"xts{q}")
                nc.sync.dma_start(tx[:], xT_r[q])
                xts_q[q] = tx

            def load_wout(g):
                tw = woutp.tile([128, JPC, D], MMD, name=f"wout{g}", tag=f"wout{g}")
                nc.sync.dma_start(tw[:], w_out_r[g])
                wout_g[g] = tw

            load_xts(0)
            load_win(0)
            load_wout(0)
            for c in range(1, NC_):
                load_win(c)
                load_wout(c)
            load_xts(1)

            # ---- PE warm-up: scratch matmuls on the first-arrived tiles
            # fill the window until `ab` is ready and bring HAM to 8/8 ----
            for w in range(16):
                pw = pap.tile([128, 128], F32, name=f"warm{w}", tag="pa")
                nc.tensor.matmul(pw[:], wm[:], wm[:], start=True, stop=True)

            # ---- fused main loop (py stage software-pipelined by one j) ----
            for q in range(NQ):
                lo, hi = q * QCH, (q + 1) * QCH
                pys = [pyp.tile([128, QCH], F32, name=f"py{q}_{j2}", tag="py")
                       for j2 in range(NK)]
                pend = None
                for j in range(NJ):
                    c, jc = j // JPC, j % JPC
                    pa = pap.tile([128, QCH], F32, tag="pa")
                    for k in range(NK):
                        nc.tensor.matmul(
                            pa[:],
                            win_c[c][:, k, 128 * jc : 128 * (jc + 1)],
                            xts_q[q][:, k, :],
                            start=(k == 0),
                            stop=(k == NK - 1),
                        )
                    pb = pbp.tile([128, QCH], F32, tag="pb")
                    nc.tensor.matmul(
                        pb[:],
                        ab[:, 128 * j : 128 * (j + 1)],
                        cs[:, lo:hi],
                        start=True,
                        stop=True,
                    )
                    msb = modsp.tile([128, QCH], F32, tag="mods")
                    nc.vector.tensor_copy(msb[:], pb[:])
                    hm = hmp.tile([128, QCH], MMD, tag="hm")
                    nc.vector.tensor_mul(hm[:], pa[:], msb[:])
                    if pend is not None:
                        phm, pj = pend
                        for j2 in range(NK):
                            nc.tensor.matmul(
                                pys[j2][:],
                                wout_g[pj // JPC][:, pj % JPC,
                                                  128 * j2 : 128 * (j2 + 1)],
                                phm[:],
                                start=(pj == 0),
                                stop=False,
                            )
                    pend = (hm, j)
                phm, pj = pend
                for j2 in range(NK):
                    nc.tensor.matmul(
                        pys[j2][:],
                        wout_g[pj // JPC][:, pj % JPC, 128 * j2 : 128 * (j2 + 1)],
                        phm[:],
                        start=(pj == 0),
                        stop=True,
                    )
                for j2 in range(NK):
                    yo = yop.tile([128, QCH], F32, tag="yo")
                    nc.scalar.copy(yo[:], pys[j2][:])
                    nc.sync.dma_start(
                        yT_d[128 * j2 : 128 * (j2 + 1), lo:hi], yo[:]
                    )

    nc.finalize()
    return nc


def _get_nc():
    if "nc" not in _cache:
        _cache["nc"] = _build()
    return _cache["nc"]


def _mmd(a):
    if USE_BF16:
        import ml_dtypes
        return np.ascontiguousarray(a.astype(ml_dtypes.bfloat16))
    return np.ascontiguousarray(a.astype(np.float32))


def _in_maps(x, input_proj, output_proj, floquet_energies, drive_weights,
             coupling_matrix):
    coupled = coupling_matrix.astype(np.float64) @ drive_weights.astype(np.float64)
    cbn = (-np.repeat(coupled, 128 // M)).astype(np.float32).reshape(128, 1)
    fe = np.ascontiguousarray(
        floquet_energies.astype(np.float32).reshape(128, M * HD // 128)
    )
    w_in = _mmd(input_proj)
    w_out = _mmd(output_proj)

    harm = np.arange(1, M + 1, dtype=np.float64)
    maps = []
    for c in range(NCORES):
        b, half = c // 2, c % 2
        t = (half * T + np.arange(T, dtype=np.float64)) / L
        ang = 2.0 * np.pi * harm[:, None] * t[None, :]
        cs_np = np.zeros((128, T), dtype=np.float64)
        cs_np[0:M] = np.cos(ang)
        cs_np[M : 2 * M] = np.sin(ang)
        cs = _mmd(cs_np)
        xT = _mmd(x[b, half * T : (half + 1) * T, :].T)
        maps.append(
            {
                "xT": xT,
                "w_in": w_in,
                "w_out": w_out,
                "cs": cs,
                "fe": fe,
                "cbn": cbn,
            }
        )
    return maps


def kernel(x, input_proj, output_proj, floquet_energies, drive_weights,
           coupling_matrix, _trace=False, _trace_kwargs=None):
    from concourse.bass_utils import run_bass_kernel_spmd

    nc = _get_nc()
    maps = _in_maps(x, input_proj, output_proj, floquet_energies,
                    drive_weights, coupling_matrix)
    kw = dict(_trace_kwargs or {})
    res = run_bass_kernel_spmd(nc, maps, list(range(NCORES)), trace=_trace, **kw)
    out = np.empty((B, L, D), dtype=np.float32)
    for c in range(NCORES):
        b, half = c // 2, c % 2
        out[b, half * T : (half + 1) * T, :] = res.results[c]["yT"].T
    if _trace:
        return out, res
    return out



# revision 4
# speedup vs baseline: 1.2233x; 1.2233x over previous
"""HDTimeCrystalBlock kernel for 8 Trainium2 NeuronCores.

Math: out = ((x @ W_in) * mod[None]) @ W_out, where
  mod[l,h] = sum_m coupled[m] * cos(omega*(m+1)*t[l] + E[m,h])
Using cos(a+b) = cos(a)cos(b) - sin(a)sin(b):
  mod = Cmat @ A + Smat @ B with A[m,h]=coupled[m]*cos(E[m,h]),
  B[m,h]=-coupled[m]*sin(E[m,h]) -> a K=2M=32 matmul per h-tile.

Sharding: data-parallel over the 8192 tokens (B*L), 1024 per core;
weights replicated; no collectives. Host precomputes A/B (tiny trig on
[32,HD]) and the cos/sin token table, both replicated 4x across
partition groups so the mod matmuls run as 4-way row-tiled packs
(K=32 each at tile_position rows 0/32/64/96 -> ~4x mod throughput).

Per q-chunk of 512 tokens the device runs two phases:
  A: per j-tile: 4 K-accumulated MM1 matmuls (hd_in) + every 4th j a
     4-way packed mod matmul; DVE multiplies PSUM(pa) x PSUM(pb) into
     a bf16 hm_all buffer.
  B: j2-outer MM2: 128 matmuls K-accumulated over j into one PSUM
     bank per j2, evacuated by ACT to bf16 and DMA'd out.
All matmuls bf16 (full PE rate); PSUM budget: 2 pa + 4 pb + 2 py = 8.
"""
import math

import numpy as np

B, L, D, HD, M = 4, 2048, 512, 4096, 16
NCORES = 8
T = (B * L) // NCORES          # tokens per core
QCH = 512                      # token chunk (PSUM bank width in fp32)
NQ = T // QCH
NJ = HD // 128                 # h-tiles
NK = D // 128                  # d-tiles
NC_ = HD // 1024               # w_in column chunks
JPC = 1024 // 128              # j-tiles per w_in chunk
PI = math.pi
NWARM = 20                     # scratch warm-up matmuls
TWO_PSUM_MUL = False           # 2-PSUM tensor_tensor rejected by walrus (NCC_IBVF027)

_cache = {}


def _build():
    from concourse import bacc, bass, mybir, tile

    F32 = mybir.dt.float32
    BF16 = mybir.dt.bfloat16
    PSUM = bass.MemorySpace.PSUM

    nc = bacc.Bacc("TRN2", target_bir_lowering=False, debug=False)

    xT_d = nc.dram_tensor("xT", [D, T], BF16, kind="ExternalInput")
    w_in_d = nc.dram_tensor("w_in", [D, HD], BF16, kind="ExternalInput")
    w_out_d = nc.dram_tensor("w_out", [HD, D], BF16, kind="ExternalInput")
    cs_d = nc.dram_tensor("cs", [128, T], BF16, kind="ExternalInput")
    ab_d = nc.dram_tensor("ab", [128, HD], BF16, kind="ExternalInput")
    yT_d = nc.dram_tensor("yT", [D, T], BF16, kind="ExternalOutput")

    with tile.TileContext(nc) as tc:
        with (
            tc.tile_pool(name="win", bufs=1) as winp,
            tc.tile_pool(name="wout", bufs=1) as woutp,
            tc.tile_pool(name="xts", bufs=1) as xtp,
            tc.tile_pool(name="small", bufs=1) as smallp,
            tc.tile_pool(name="hma", bufs=1) as hmap,
            tc.tile_pool(name="mods", bufs=3) as modsp,
            tc.tile_pool(name="yo", bufs=4) as yop,
            tc.tile_pool(name="pa", bufs=2, space=PSUM) as pap,
            tc.tile_pool(name="pb", bufs=4, space=PSUM) as pbp,
            tc.tile_pool(name="py", bufs=2, space=PSUM) as pyp,
        ):
            # ---- small gating inputs first ----
            cs = smallp.tile([128, T], BF16, tag="cs")
            ab = smallp.tile([128, HD], BF16, tag="ab")
            nc.sync.dma_start(cs[:], cs_d[:])
            nc.sync.dma_start(ab[:], ab_d[:])

            # ---- bulk loads, spread across engine queues so descriptor
            # generation parallelizes and the PE never starves ----
            w_in_r = w_in_d.ap().rearrange("(k p) (c h) -> c p k h", p=128, c=NC_)
            xT_r = xT_d.ap().rearrange("(k p) (q t) -> q p k t", p=128, q=NQ)
            w_out_r = w_out_d.ap().rearrange("(g jj p) i -> g p jj i", p=128, jj=JPC)

            win_c = [None] * NC_
            xts_q = [None] * NQ
            wout_g = [None] * NC_

            def load_win(eng, c):
                t_ = winp.tile([128, NK, 1024], BF16, name=f"win{c}", tag=f"win{c}")
                eng.dma_start(t_[:], w_in_r[c])
                win_c[c] = t_

            def load_xts(eng, q):
                tx = xtp.tile([128, NK, QCH], BF16, name=f"xts{q}", tag=f"xts{q}")
                eng.dma_start(tx[:], xT_r[q])
                xts_q[q] = tx

            def load_wout(eng, g):
                tw = woutp.tile([128, JPC, D], BF16, name=f"wout{g}", tag=f"wout{g}")
                eng.dma_start(tw[:], w_out_r[g])
                wout_g[g] = tw

            load_xts(nc.gpsimd, 0)
            load_win(nc.sync, 0)
            load_win(nc.gpsimd, 1)
            load_wout(nc.scalar, 0)
            load_win(nc.sync, 2)
            load_win(nc.gpsimd, 3)
            load_wout(nc.scalar, 1)
            load_wout(nc.sync, 2)
            load_wout(nc.gpsimd, 3)
            load_xts(nc.scalar, 1)

            # ---- PE warm-up on cs: brings HAM to 8/8 while bulk DMAs land ----
            for w in range(NWARM):
                pw = pyp.tile([128, QCH], F32, name=f"warm{w}", tag="py")
                nc.tensor.matmul(pw[:], cs[:, 0:128], cs[:, 0:QCH],
                                 start=True, stop=True)

            hm_all = [hmap.tile([128, NJ, QCH], BF16, name=f"hma{q}", tag=f"hma{q}")
                      for q in range(NQ)]

            for q in range(NQ):
                lo, hi = q * QCH, (q + 1) * QCH
                # ---- phase A: hd_in tiles + packed mod, fused into hm ----
                pbt = [None] * 4
                for j in range(NJ):
                    if j % 4 == 0:
                        # 4-way row-tiled mod pack: tile i computes j+i
                        for i in range(4):
                            pb = pbp.tile([128, QCH], F32, tag="pb")
                            nc.tensor.matmul(
                                pb[:],
                                ab[32 * i : 32 * (i + 1),
                                   128 * (j + i) : 128 * (j + i + 1)],
                                cs[32 * i : 32 * (i + 1), lo:hi],
                                start=True,
                                stop=True,
                                tile_position=(32 * i, 0),
                            )
                            pbt[i] = pb
                    pa = pap.tile([128, QCH], F32, tag="pa")
                    c, jc = j // JPC, j % JPC
                    for k in range(NK):
                        nc.tensor.matmul(
                            pa[:],
                            win_c[c][:, k, 128 * jc : 128 * (jc + 1)],
                            xts_q[q][:, k, :],
                            start=(k == 0),
                            stop=(k == NK - 1),
                        )
                    if TWO_PSUM_MUL:
                        nc.vector.tensor_mul(hm_all[q][:, j, :], pa[:],
                                             pbt[j % 4][:])
                    else:
                        msb = modsp.tile([128, QCH], F32, tag="mods")
                        nc.scalar.copy(msb[:], pbt[j % 4][:])
                        nc.vector.tensor_mul(hm_all[q][:, j, :], pa[:], msb[:])

                # ---- phase B: j2-outer output projection ----
                for j2 in range(NK):
                    py = pyp.tile([128, QCH], F32, tag="py")
                    for j in range(NJ):
                        nc.tensor.matmul(
                            py[:],
                            wout_g[j // JPC][:, j % JPC,
                                             128 * j2 : 128 * (j2 + 1)],
                            hm_all[q][:, j, :],
                            start=(j == 0),
                            stop=(j == NJ - 1),
                        )
                    yo = yop.tile([128, QCH], BF16, tag="yo")
                    nc.scalar.copy(yo[:], py[:])
                    eng = (nc.sync, nc.gpsimd)[j2 % 2]
                    eng.dma_start(yT_d[128 * j2 : 128 * (j2 + 1), lo:hi], yo[:])

    nc.finalize()
    return nc


def _get_nc():
    if "nc" not in _cache:
        _cache["nc"] = _build()
    return _cache["nc"]


def _bf(a):
    import ml_dtypes
    return np.ascontiguousarray(np.asarray(a, dtype=np.float32).astype(ml_dtypes.bfloat16))


def _in_maps(x, input_proj, output_proj, floquet_energies, drive_weights,
             coupling_matrix):
    coupled = coupling_matrix.astype(np.float64) @ drive_weights.astype(np.float64)
    # ab rows 0:16 = coupled*cos(E), rows 16:32 = -coupled*sin(E),
    # replicated into partition groups 0/32/64/96 for 4-way row tiling
    E = floquet_energies.astype(np.float64)
    ab32 = np.concatenate(
        [coupled[:, None] * np.cos(E), -coupled[:, None] * np.sin(E)], axis=0
    )
    ab = _bf(np.tile(ab32, (4, 1)))

    w_in = _bf(input_proj)
    w_out = _bf(output_proj)

    harm = np.arange(1, M + 1, dtype=np.float64)
    maps = []
    for c in range(NCORES):
        b, half = c // 2, c % 2
        t = (half * T + np.arange(T, dtype=np.float64)) / L
        ang = 2.0 * np.pi * harm[:, None] * t[None, :]
        cs32 = np.concatenate([np.cos(ang), np.sin(ang)], axis=0)
        cs = _bf(np.tile(cs32, (4, 1)))
        xT = _bf(x[b, half * T : (half + 1) * T, :].T)
        maps.append(
            {"xT": xT, "w_in": w_in, "w_out": w_out, "cs": cs, "ab": ab}
        )
    return maps


def kernel(x, input_proj, output_proj, floquet_energies, drive_weights,
           coupling_matrix, _trace=False, _trace_kwargs=None):
    from concourse.bass_utils import run_bass_kernel_spmd

    nc = _get_nc()
    maps = _in_maps(x, input_proj, output_proj, floquet_energies,
                    drive_weights, coupling_matrix)
    kw = dict(_trace_kwargs or {})
    res = run_bass_kernel_spmd(nc, maps, list(range(NCORES)), trace=_trace, **kw)
    out = np.empty((B, L, D), dtype=np.float32)
    for c in range(NCORES):
        b, half = c // 2, c % 2
        out[b, half * T : (half + 1) * T, :] = \
            res.results[c]["yT"].T.astype(np.float32)
    if _trace:
        return out, res
    return out


# revision 8
# speedup vs baseline: 1.2491x; 1.0211x over previous
"""HDTimeCrystalBlock kernel for 8 Trainium2 NeuronCores.

Math: out = ((x @ W_in) * mod[None]) @ W_out, where
  mod[l,h] = sum_m coupled[m] * cos(omega*(m+1)*t[l] + E[m,h])
Using cos(a+b) = cos(a)cos(b) - sin(a)sin(b):
  mod = Cmat @ A + Smat @ B with A[m,h]=coupled[m]*cos(E[m,h]),
  B[m,h]=-coupled[m]*sin(E[m,h]) -> a K=2M=32 matmul per h-tile.

Sharding: data-parallel over the 8192 tokens (B*L), 1024 per core;
weights replicated; no collectives. Host precomputes A/B (tiny trig on
[32,HD]) and the cos/sin token table, both replicated 4x across
partition groups so the mod matmuls run as 4-way row-tiled packs
(K=32 each at tile_position rows 0/32/64/96 -> ~4x mod throughput).

Per q-chunk of 512 tokens the device runs two phases:
  A: per j-tile: 4 K-accumulated MM1 matmuls (hd_in) + every 4th j a
     4-way packed mod matmul; DVE multiplies PSUM(pa) x PSUM(pb) into
     a bf16 hm_all buffer.
  B: j2-outer MM2: 128 matmuls K-accumulated over j into one PSUM
     bank per j2, evacuated by ACT to bf16 and DMA'd out.
All matmuls bf16 (full PE rate); PSUM budget: 2 pa + 4 pb + 2 py = 8.
"""
import math

import numpy as np

B, L, D, HD, M = 4, 2048, 512, 4096, 16
NCORES = 8
T = (B * L) // NCORES          # tokens per core
QCH = 512                      # token chunk (PSUM bank width in fp32)
NQ = T // QCH
NJ = HD // 128                 # h-tiles
NK = D // 128                  # d-tiles
NC_ = HD // 1024               # w_in column chunks
JPC = 1024 // 128              # j-tiles per w_in chunk
PI = math.pi
NWARM = 10                     # scratch warm-up matmuls

_cache = {}


def _build():
    from concourse import bacc, bass, mybir, tile

    F32 = mybir.dt.float32
    BF16 = mybir.dt.bfloat16
    PSUM = bass.MemorySpace.PSUM

    nc = bacc.Bacc("TRN2", target_bir_lowering=False, debug=False)

    xT_d = nc.dram_tensor("xT", [D, T], BF16, kind="ExternalInput")
    w_in_d = nc.dram_tensor("w_in", [D, HD], BF16, kind="ExternalInput")
    w_out_d = nc.dram_tensor("w_out", [HD, D], BF16, kind="ExternalInput")
    cs_d = nc.dram_tensor("cs", [128, T], BF16, kind="ExternalInput")
    ab_d = nc.dram_tensor("ab", [128, HD], BF16, kind="ExternalInput")
    yT_d = nc.dram_tensor("yT", [D, T], BF16, kind="ExternalOutput")

    with tile.TileContext(nc) as tc:
        with (
            tc.tile_pool(name="win", bufs=1) as winp,
            tc.tile_pool(name="wout", bufs=1) as woutp,
            tc.tile_pool(name="xts", bufs=1) as xtp,
            tc.tile_pool(name="small", bufs=1) as smallp,
            tc.tile_pool(name="hma", bufs=1) as hmap,
            tc.tile_pool(name="mods", bufs=3) as modsp,
            tc.tile_pool(name="yo", bufs=4) as yop,
            tc.tile_pool(name="pa", bufs=2, space=PSUM) as pap,
            tc.tile_pool(name="pb", bufs=4, space=PSUM) as pbp,
            tc.tile_pool(name="py", bufs=2, space=PSUM) as pyp,
        ):
            # ---- small gating inputs first ----
            cs = smallp.tile([128, T], BF16, tag="cs")
            ab = smallp.tile([128, HD], BF16, tag="ab")
            nc.sync.dma_start(cs[:], cs_d[:])
            nc.gpsimd.dma_start(ab[:], ab_d[:])

            # ---- bulk loads, ordered by first-use time and spread across
            # engine queues; the c=0 / q=0 chunks that gate the first MM1s
            # are split per k-slice so the PE starts on partial data ----
            w_in_r = w_in_d.ap().rearrange("(k p) (c h) -> c p k h", p=128, c=NC_)
            xT_r = xT_d.ap().rearrange("(k p) (q t) -> q p k t", p=128, q=NQ)
            w_out_r = w_out_d.ap().rearrange("(g jj p) i -> g p jj i", p=128, jj=JPC)

            win_c = [None] * NC_
            xts_q = [None] * NQ
            wout_g = [None] * NC_
            win0k = [None] * NK
            xts0k = [None] * NK

            def load_win0k(eng, k):
                t_ = winp.tile([128, 1024], BF16, name=f"win0k{k}", tag=f"win0k{k}")
                eng.dma_start(t_[:], w_in_r[0][:, k, :])
                win0k[k] = t_

            def load_xts0k(eng, k):
                tx = xtp.tile([128, QCH], BF16, name=f"xts0k{k}", tag=f"xts0k{k}")
                eng.dma_start(tx[:], xT_r[0][:, k, :])
                xts0k[k] = tx

            def load_win(eng, c):
                t_ = winp.tile([128, NK, 1024], BF16, name=f"win{c}", tag=f"win{c}")
                eng.dma_start(t_[:], w_in_r[c])
                win_c[c] = t_

            def load_xts(eng, q):
                tx = xtp.tile([128, NK, QCH], BF16, name=f"xts{q}", tag=f"xts{q}")
                eng.dma_start(tx[:], xT_r[q])
                xts_q[q] = tx

            def load_wout(eng, g):
                tw = woutp.tile([128, JPC, D], BF16, name=f"wout{g}", tag=f"wout{g}")
                eng.dma_start(tw[:], w_out_r[g])
                wout_g[g] = tw

            load_xts0k(nc.scalar, 0)
            load_win0k(nc.sync, 0)
            load_xts0k(nc.gpsimd, 1)
            load_win0k(nc.scalar, 1)
            load_xts0k(nc.sync, 2)
            load_win0k(nc.gpsimd, 2)
            load_xts0k(nc.scalar, 3)
            load_win0k(nc.sync, 3)
            load_win(nc.gpsimd, 1)
            load_win(nc.sync, 2)
            load_win(nc.scalar, 3)
            load_wout(nc.scalar, 0)
            load_wout(nc.sync, 1)
            load_wout(nc.gpsimd, 2)
            load_wout(nc.scalar, 3)
            load_xts(nc.gpsimd, 1)

            # ---- PE warm-up on cs: brings HAM to 8/8 while bulk DMAs land ----
            for w in range(NWARM):
                pw = pyp.tile([128, QCH], F32, name=f"warm{w}", tag="py")
                nc.tensor.matmul(pw[:], cs[:, 0:128], cs[:, 0:QCH],
                                 start=True, stop=True)

            hm_all = [hmap.tile([128, NJ, QCH], BF16, name=f"hma{q}", tag=f"hma{q}")
                      for q in range(NQ)]

            def mm1_ops(q, j, k):
                c, jc = j // JPC, j % JPC
                lw = (win0k[k][:, 128 * jc : 128 * (jc + 1)] if c == 0
                      else win_c[c][:, k, 128 * jc : 128 * (jc + 1)])
                rx = xts0k[k][:] if q == 0 else xts_q[q][:, k, :]
                return lw, rx

            def mod_pack(j0, lo, hi, pbt):
                # 4-way row-tiled mod pack: tile i computes j0+i
                for i in range(4):
                    pb = pbp.tile([128, QCH], F32, tag="pb")
                    nc.tensor.matmul(
                        pb[:],
                        ab[32 * i : 32 * (i + 1),
                           128 * (j0 + i) : 128 * (j0 + i + 1)],
                        cs[32 * i : 32 * (i + 1), lo:hi],
                        start=True,
                        stop=True,
                        tile_position=(32 * i, 0),
                    )
                    pbt[i] = pb

            for q in range(NQ):
                lo, hi = q * QCH, (q + 1) * QCH
                # ---- phase A: hd_in tiles + packed mod, fused into hm ----
                pbt = [None] * 4
                for j in range(NJ):
                    pa = pap.tile([128, QCH], F32, tag="pa")
                    for k in range(NK):
                        lw, rx = mm1_ops(q, j, k)
                        nc.tensor.matmul(pa[:], lw, rx,
                                         start=(k == 0), stop=(k == NK - 1))
                    # mod pack sits after the first MM1 group of each 4-j
                    # block so the first MM1s aren't gated on the ab DMA
                    if j % 4 == 0:
                        mod_pack(j, lo, hi, pbt)
                    msb = modsp.tile([128, QCH], F32, tag="mods")
                    nc.scalar.copy(msb[:], pbt[j % 4][:])
                    nc.vector.tensor_mul(hm_all[q][:, j, :], pa[:], msb[:])

                # ---- phase B: j2-outer output projection ----
                for j2 in range(NK):
                    py = pyp.tile([128, QCH], F32, tag="py")
                    for j in range(NJ):
                        nc.tensor.matmul(
                            py[:],
                            wout_g[j // JPC][:, j % JPC,
                                             128 * j2 : 128 * (j2 + 1)],
                            hm_all[q][:, j, :],
                            start=(j == 0),
                            stop=(j == NJ - 1),
                        )
                    # evacuate on ACT + DVE halves in parallel
                    yo = yop.tile([128, QCH], BF16, tag="yo")
                    nc.scalar.copy(yo[:, 0:QCH // 2], py[:, 0:QCH // 2])
                    nc.vector.tensor_copy(yo[:, QCH // 2 :], py[:, QCH // 2 :])
                    eng = (nc.sync, nc.gpsimd)[j2 % 2]
                    eng.dma_start(yT_d[128 * j2 : 128 * (j2 + 1), lo:hi], yo[:])

    nc.finalize()
    return nc


def _get_nc():
    if "nc" not in _cache:
        _cache["nc"] = _build()
    return _cache["nc"]


def _bf(a):
    import ml_dtypes
    return np.ascontiguousarray(np.asarray(a, dtype=np.float32).astype(ml_dtypes.bfloat16))


def _in_maps(x, input_proj, output_proj, floquet_energies, drive_weights,
             coupling_matrix):
    coupled = coupling_matrix.astype(np.float64) @ drive_weights.astype(np.float64)
    # ab rows 0:16 = coupled*cos(E), rows 16:32 = -coupled*sin(E),
    # replicated into partition groups 0/32/64/96 for 4-way row tiling
    E = floquet_energies.astype(np.float64)
    ab32 = np.concatenate(
        [coupled[:, None] * np.cos(E), -coupled[:, None] * np.sin(E)], axis=0
    )
    ab = _bf(np.tile(ab32, (4, 1)))

    w_in = _bf(input_proj)
    w_out = _bf(output_proj)

    harm = np.arange(1, M + 1, dtype=np.float64)
    maps = []
    for c in range(NCORES):
        b, half = c // 2, c % 2
        t = (half * T + np.arange(T, dtype=np.float64)) / L
        ang = 2.0 * np.pi * harm[:, None] * t[None, :]
        cs32 = np.concatenate([np.cos(ang), np.sin(ang)], axis=0)
        cs = _bf(np.tile(cs32, (4, 1)))
        xT = _bf(x[b, half * T : (half + 1) * T, :].T)
        maps.append(
            {"xT": xT, "w_in": w_in, "w_out": w_out, "cs": cs, "ab": ab}
        )
    return maps


def kernel(x, input_proj, output_proj, floquet_energies, drive_weights,
           coupling_matrix, _trace=False, _trace_kwargs=None):
    from concourse.bass_utils import run_bass_kernel_spmd

    nc = _get_nc()
    maps = _in_maps(x, input_proj, output_proj, floquet_energies,
                    drive_weights, coupling_matrix)
    kw = dict(_trace_kwargs or {})
    res = run_bass_kernel_spmd(nc, maps, list(range(NCORES)), trace=_trace, **kw)
    out = np.empty((B, L, D), dtype=np.float32)
    for c in range(NCORES):
        b, half = c // 2, c % 2
        out[b, half * T : (half + 1) * T, :] = \
            res.results[c]["yT"].T.astype(np.float32)
    if _trace:
        return out, res
    return out
